# revision 1
# baseline (speedup 1.0000x reference)
# Bahdanau (content-based additive) attention kernel for Trainium2.
#
# reference computation (B=64, S=2048, H=512, fp32):
#   query  = decoder_h_t.transpose(1,0,2)                  # [B,1,H]
#   proj_q = query @ W_h.T                                 # [B,1,H]
#   proj_k = keys  @ W_s.T + b_s                           # [B,S,H]
#   energy = tanh(proj_q + proj_k) @ v                     # [B,S]
#   wgt    = softmax(energy, axis=1)
#   ctx    = wgt[:,None,:] @ keys                          # [B,1,H]
#
# Distribution: pure data-parallel over batch. 8 NeuronCores x 8 batches.
# Weights replicated. No collectives; gather on host.
#
# Per-core dataflow (per batch b, s-tile t of 512):
#   PE  : proj^T[o,s] = sum_h WsT[h,o] * KT[h,s]   (KT kept [h,s] in SBUF, bf16)
#   ACT : tanh(proj + (W_h q + b_s)[o] per-partition bias) -> SBUF bf16
#   PE  : energy[1,s] = sum_o v[o] * tanh[o,s]     (M=1 accumulating matmuls)
#   ACT : eexp = exp(energy) (bf16) with accum_out -> per-tile sum (f32)
#   DVE : ctx_unnorm[h] = sum_s KT[h,s] * eexp[s]  (tensor_tensor_reduce)
#   DVE : ctx = ctx_unnorm / sum(eexp)  (scale by ACT-reciprocal)
# Final [128,32] -> [32,128] via DVE stream transposes, one clean DMA out.

import sys

import numpy as np

try:
    import concourse.bacc as bacc  # noqa: F401
except Exception:  # pragma: no cover - fresh grading dir fallback
    sys.path.insert(0, "/opt/trn_rl_repo")

import ml_dtypes

import concourse.bacc as bacc
import concourse.bass as bass
import concourse.mybir as mybir
import concourse.tile as tile
from concourse.bass_utils import run_bass_kernel_spmd

N_CORES = 8
B, S, H = 64, 2048, 512
B_PC = B // N_CORES  # batches per core

F32 = mybir.dt.float32
BF16 = mybir.dt.bfloat16
AF = mybir.ActivationFunctionType
ALU = mybir.AluOpType

ST = 512  # s-tile (matmul moving free dim)


def build_nc(b_pc: int = B_PC, s: int = S, h: int = H) -> bacc.Bacc:
    HC = h // 128  # contraction chunks (h on partitions)
    OC = h // 128  # output-row chunks (o on partitions)
    NT = s // ST   # s tiles

    nc = bacc.Bacc(
        "TRN2",
        target_bir_lowering=False,
        debug=False,
        enable_asserts=False,
    )

    kt_d = nc.dram_tensor("kt", [b_pc, h, s], BF16, kind="ExternalInput").ap()
    wst_d = nc.dram_tensor("wst", [h, h], BF16, kind="ExternalInput").ap()
    wht_d = nc.dram_tensor("wht", [h, h], F32, kind="ExternalInput").ap()
    qt_d = nc.dram_tensor("qt", [h, b_pc], F32, kind="ExternalInput").ap()
    bs_d = nc.dram_tensor("bs", [h, 1], F32, kind="ExternalInput").ap()
    # v replicated 128x along columns: row-broadcast energies come out of PE
    vv_d = nc.dram_tensor("vv", [h, 128], BF16, kind="ExternalInput").ap()
    out_d = nc.dram_tensor("ctx", [b_pc, h], F32, kind="ExternalOutput").ap()

    with tile.TileContext(nc) as tc:
        with (
            tc.tile_pool(name="const", bufs=1) as constp,
            tc.tile_pool(name="work", bufs=2) as workp,
            tc.tile_pool(name="psA", bufs=6, space="PSUM") as psA,
            tc.tile_pool(name="psB", bufs=2, space="PSUM") as psB,
        ):
            # ---- constants -------------------------------------------------
            wst_sb = []
            wht_sb = []
            qt_sb = []
            bs_sb = []
            vv_sb = []
            # wst first: it gates the very first proj matmul
            for hc in range(HC):
                t1 = constp.tile([128, h], BF16, name=f"wst{hc}", tag=f"wst{hc}")
                nc.sync.dma_start(t1[:], wst_d[hc * 128:(hc + 1) * 128, :])
                wst_sb.append(t1)
            # batch-0 K^T tiles next (see below) are issued before the f32
            # q-side constants, which are only needed for the tanh bias.
            early_kt = []
            for hc in range(HC):
                kt_t = constp.tile(
                    [128, s], BF16, name=f"kt0_{hc}", tag=f"kt0_{hc}"
                )
                nc.sync.dma_start(kt_t[:], kt_d[0, hc * 128:(hc + 1) * 128, :])
                early_kt.append(kt_t)
            for hc in range(HC):
                t2 = constp.tile([128, h], F32, name=f"wht{hc}", tag=f"wht{hc}")
                nc.sync.dma_start(t2[:], wht_d[hc * 128:(hc + 1) * 128, :])
                wht_sb.append(t2)
                t3 = constp.tile([128, b_pc], F32, name=f"qt{hc}", tag=f"qt{hc}")
                nc.sync.dma_start(t3[:], qt_d[hc * 128:(hc + 1) * 128, :])
                qt_sb.append(t3)
                t4 = constp.tile([128, 1], F32, name=f"bs{hc}", tag=f"bs{hc}")
                nc.sync.dma_start(t4[:], bs_d[hc * 128:(hc + 1) * 128, :])
                bs_sb.append(t4)
                t5 = constp.tile([128, 128], BF16, name=f"vv{hc}", tag=f"vv{hc}")
                nc.sync.dma_start(t5[:], vv_d[hc * 128:(hc + 1) * 128, :])
                vv_sb.append(t5)

            # ---- q-side bias: qbias[o, b] = (W_h q)[o, b] + b_s[o] ---------
            qbias_sb = []
            for oc in range(OC):
                pq = psB.tile([128, max(b_pc, 1)], F32, name=f"pq{oc}", tag="small")
                for hc in range(HC):
                    nc.tensor.matmul(
                        pq[:],
                        wht_sb[hc][:, oc * 128:(oc + 1) * 128],
                        qt_sb[hc][:],
                        start=(hc == 0),
                        stop=(hc == HC - 1),
                    )
                qb = constp.tile([128, b_pc], F32, name=f"qbias{oc}", tag=f"qbias{oc}")
                nc.vector.tensor_scalar_add(qb[:], pq[:], bs_sb[oc][:])
                qbias_sb.append(qb)

            # ---- resident K^T tiles ---------------------------------------
            kt_sb = [[None] * HC for _ in range(b_pc)]
            kt_sb[0] = early_kt
            for b in range(1, b_pc):
                for hc in range(HC):
                    kt_t = constp.tile(
                        [128, s], BF16, name=f"kt{b}_{hc}", tag=f"kt{b}_{hc}"
                    )
                    nc.sync.dma_start(kt_t[:], kt_d[b, hc * 128:(hc + 1) * 128, :])
                    kt_sb[b][hc] = kt_t

            # ---- outputs accumulators -------------------------------------
            n_cols = b_pc * OC  # 32 at full size
            assert n_cols <= 32
            cun_sb = constp.tile([128, b_pc * OC], F32, name="cun", tag="cun")
            ctxs_sb = constp.tile([128, 32], F32, name="ctxs", tag="ctxs")
            ctxT_sb = constp.tile([32, 128], F32, name="ctxT", tag="ctxT")
            if n_cols < 32:
                nc.vector.memset(ctxs_sb[:], 0.0)
            rr_sb = [
                constp.tile([128, 1], F32, name=f"rr{b}", tag=f"rr{b}")
                for b in range(b_pc)
            ]

            # ---- main loop (v-reduce pipelined one s-tile behind proj) -----
            def emit_vred_exp(pb, pt, ptanh, peexp, pesum):
                eps = psB.tile([128, ST], F32, name=f"en{pb}_{pt}", tag="small")
                for oc in range(OC):
                    nc.tensor.matmul(
                        eps[:], vv_sb[oc][:], ptanh[oc][:],
                        start=(oc == 0), stop=(oc == OC - 1),
                    )
                nc.scalar.activation(
                    peexp[:, pt * ST:(pt + 1) * ST], eps[:], AF.Exp,
                    accum_out=pesum[:, pt:pt + 1],
                )

            def emit_softmax_ctx(pb, peexp, pesum):
                ssum = workp.tile([128, 1], F32, name=f"ssum{pb}", tag="ssum")
                nc.vector.tensor_reduce(
                    ssum[:], pesum[:], axis=mybir.AxisListType.X, op=ALU.add
                )
                nc.vector.reciprocal(rr_sb[pb][:], ssum[:])
                # context: cun[h, pb*OC+hc] = sum_s KT[h,s] * eexp[s]
                for hc in range(HC):
                    junk = workp.tile(
                        [128, s], BF16, name=f"junk{pb}_{hc}", tag="junk"
                    )
                    # fused multiply + free-dim sum: out=(in0*1)*in1, accum=sum
                    nc.vector.scalar_tensor_tensor(
                        out=junk[:],
                        in0=kt_sb[pb][hc][:],
                        scalar=1.0,
                        in1=peexp[:],
                        op0=ALU.mult,
                        op1=ALU.mult,
                        accum_out=cun_sb[:, pb * OC + hc: pb * OC + hc + 1],
                    )
                nc.vector.tensor_scalar_mul(
                    ctxs_sb[:, pb * OC:(pb + 1) * OC],
                    cun_sb[:, pb * OC:(pb + 1) * OC],
                    rr_sb[pb][:],
                )

            pending = None
            for b in range(b_pc):
                eexp_b = workp.tile([128, s], BF16, name=f"eexp{b}", tag="eexp")
                esum_b = workp.tile([128, NT], F32, name=f"esum{b}", tag="esum")
                for t in range(NT):
                    tanh_tiles = []
                    for oc in range(OC):
                        ps = psA.tile([128, ST], F32, name=f"proj{b}_{t}_{oc}", tag="proj")
                        for hc in range(HC):
                            nc.tensor.matmul(
                                ps[:],
                                wst_sb[hc][:, oc * 128:(oc + 1) * 128],
                                kt_sb[b][hc][:, t * ST:(t + 1) * ST],
                                start=(hc == 0),
                                stop=(hc == HC - 1),
                            )
                        th = workp.tile(
                            [128, ST], BF16, name=f"tanh{b}_{t}_{oc}", tag="tanh",
                            bufs=8,
                        )
                        nc.scalar.activation(
                            th[:], ps[:], AF.Tanh,
                            bias=qbias_sb[oc][:, b:b + 1], scale=1.0,
                        )
                        tanh_tiles.append(th)
                    # flush previous tile's v-reduce now: its tanh had a full
                    # s-tile of PE time to finish on ACT -> no PE stall
                    if pending is not None:
                        pb, pt, ptanh, peexp, pesum = pending
                        emit_vred_exp(pb, pt, ptanh, peexp, pesum)
                        if pt == NT - 1:
                            emit_softmax_ctx(pb, peexp, pesum)
                    pending = (b, t, tanh_tiles, eexp_b, esum_b)
            pb, pt, ptanh, peexp, pesum = pending
            emit_vred_exp(pb, pt, ptanh, peexp, pesum)
            emit_softmax_ctx(pb, peexp, pesum)

            # ---- [128, 32] -> [32, 128] and store -------------------------
            for i in range(0, 128, 32):
                nc.vector.transpose(
                    ctxT_sb[0:32, i:i + 32], ctxs_sb[i:i + 32, 0:32]
                )
            out_view = out_d.rearrange("b (c j) -> (b c) j", j=128)
            nc.sync.dma_start(out_view, ctxT_sb[0:n_cols, :])

    nc.compile()
    return nc


_NC_CACHE: dict = {}


def _get_nc() -> bacc.Bacc:
    if "nc" not in _NC_CACHE:
        _NC_CACHE["nc"] = build_nc()
    return _NC_CACHE["nc"]


def make_in_maps(
    encoder_outputs: np.ndarray,
    decoder_h_t: np.ndarray,
    W_h: np.ndarray,
    W_s: np.ndarray,
    b_s: np.ndarray,
    v: np.ndarray,
) -> list[dict[str, np.ndarray]]:
    bf = ml_dtypes.bfloat16
    enc_t = np.ascontiguousarray(
        np.asarray(encoder_outputs, np.float32).transpose(0, 2, 1)
    ).astype(bf)  # [B, H, S]
    wst = np.ascontiguousarray(np.asarray(W_s, np.float32).T).astype(bf)
    wht = np.ascontiguousarray(np.asarray(W_h, np.float32).T).astype(np.float32)
    qt = np.ascontiguousarray(np.asarray(decoder_h_t, np.float32)[0].T)  # [H, B]
    bs = np.asarray(b_s, np.float32).reshape(H, 1).copy()
    vv = np.ascontiguousarray(
        np.tile(np.asarray(v, np.float32).reshape(H, 1), (1, 128))
    ).astype(bf)
    in_maps = []
    for c in range(N_CORES):
        sl = slice(c * B_PC, (c + 1) * B_PC)
        in_maps.append(
            {
                "kt": enc_t[sl],
                "wst": wst,
                "wht": wht,
                "qt": np.ascontiguousarray(qt[:, sl]),
                "bs": bs,
                "vv": vv,
            }
        )
    return in_maps


def run_traced(inputs: dict, trace: bool = False, **kw):
    nc = _get_nc()
    in_maps = make_in_maps(
        inputs["encoder_outputs"], inputs["decoder_h_t"], inputs["W_h"],
        inputs["W_s"], inputs["b_s"], inputs["v"],
    )
    res = run_bass_kernel_spmd(
        nc, in_maps, core_ids=list(range(N_CORES)), trace=trace, **kw
    )
    ctx = np.concatenate(
        [np.asarray(res.results[c]["ctx"], np.float32) for c in range(N_CORES)],
        axis=0,
    )  # [B, H]
    return ctx[:, None, :], res  # [B, 1, H]


def kernel(
    encoder_outputs: np.ndarray,
    decoder_h_t: np.ndarray,
    W_h: np.ndarray,
    W_s: np.ndarray,
    b_s: np.ndarray,
    v: np.ndarray,
) -> np.ndarray:
    out, _ = run_traced(
        {
            "encoder_outputs": encoder_outputs,
            "decoder_h_t": decoder_h_t,
            "W_h": W_h,
            "W_s": W_s,
            "b_s": b_s,
            "v": v,
        }
    )
    return out

